# revision 3
# baseline (speedup 1.0000x reference)
"""Two-layer GCN (PyG GCNConv x2 + ReLU) on 8 Trainium2 NeuronCores — v2.

Strategy (dst-sharded, SPMD, bf16):
  - Nodes padded to 102400, degree-balance-dealt to positions, sharded
    12800/core by destination. Table rows are bf16 (256B gathers).
  - Symmetric norm a_src*a_dst is split: a_src is folded into the gathered
    table (dense epilogue scales row d by a_d), a_dst is deferred through
    ReLU (b=0) and applied as a per-partition scale at the next dense /
    final epilogue. The edge selection matrix S is then pure 0/1 and is
    built ON DEVICE by DVE is_equal(iota, dloc) — no S DMA traffic at all.
  - Self-loops never touch the gather path: layer 1 opens each dst block's
    PSUM group with matmul(lhsT=W1, rhs=xT2) where xT2 = x^T * a (host
    prescaled); layer 2 adds the resident g2 tile (== a_d * h2[d]) at the
    epilogue.
  - Edge schedule: stages of 8 dst blocks; per stage one gather call per
    source window (4 windows of 25600 rows for int16 indices). Each dst
    block accumulates in a single PSUM group (no partial merges).
  - Layer 1 accumulates transposed (psum[f, d]) so its ReLU output is
    directly the lhsT for layer 2's dense matmul; layer 2 accumulates
    normal (psum[d, f]) so the final output is node-row-major.
"""

import numpy as np

import concourse.bass as bass
import concourse.bacc as bacc
import concourse.mybir as mybir
import concourse.tile as tile
from concourse.bass_utils import run_bass_kernel_spmd

N = 100000
E = 640000
D = 128
NCORES = 8
NPAD = 102400
SHARD = NPAD // NCORES        # 12800
NBLK = SHARD // 128           # 100 dst blocks per core
WIN = 25600                   # gather window rows (int16-safe)
NW = NPAD // WIN              # 4 windows
SPB = 4                       # dst blocks per stage (PSUM groups in flight)
NSTG = (NBLK + SPB - 1) // SPB
CHUNK_T = 8                   # max tiles (128 edges) per dma_gather call

BF16 = mybir.dt.bfloat16
F32 = mybir.dt.float32
NPBF16 = mybir.dt.np(BF16)

_CACHE = {}


def _host_prep(x, edge_index, W1, b1, W2, b2):
    x = np.asarray(x, dtype=np.float32)
    ei = np.asarray(edge_index)
    W1 = np.asarray(W1, dtype=np.float32)
    W2 = np.asarray(W2, dtype=np.float32)
    b1 = np.asarray(b1, dtype=np.float32)
    b2 = np.asarray(b2, dtype=np.float32)
    n = x.shape[0]
    assert (n, x.shape[1]) == (N, D) and ei.shape[1] == E

    src = ei[0].astype(np.int64)
    dst = ei[1].astype(np.int64)
    deg = np.bincount(np.concatenate([dst, np.arange(n)]), minlength=NPAD)
    deg = deg.astype(np.float32)
    a = np.zeros(NPAD, np.float32)
    nz = deg > 0
    a[nz] = 1.0 / np.sqrt(deg[nz])

    bias_mode = bool(np.any(b1 != 0.0) or np.any(b2 != 0.0))

    # degree-balanced node->position permutation (positions core-major)
    order_by_deg = np.argsort(-deg, kind="stable")
    i = np.arange(NPAD, dtype=np.int64)
    cb = i % (NCORES * NBLK)
    position_of_rank = ((cb % NCORES) * SHARD + (cb // NCORES) * 128
                        + i // (NCORES * NBLK))
    pos_of_node = np.empty(NPAD, np.int64)
    pos_of_node[order_by_deg] = position_of_rank
    node_at_pos = np.empty(NPAD, np.int64)
    node_at_pos[pos_of_node] = np.arange(NPAD, dtype=np.int64)
    a_pos = a[node_at_pos]                       # a in position space

    ps = pos_of_node[src]
    pd = pos_of_node[dst]
    core = pd // SHARD
    w_all = ps // WIN
    b_all = (pd % SHARD) // 128                  # dst block within core
    s_all = b_all // SPB                         # stage
    j_all = b_all % SPB                          # block within stage
    # group key in schedule order: (stage, window, block-within-stage)
    key_all = (s_all * NW + w_all) * SPB + j_all
    NG = NSTG * NW * SPB

    counts_all = np.zeros((NCORES, NG), np.int64)
    per_core = []
    for k in range(NCORES):
        m = core == k
        s_k, d_k, key_k = ps[m], pd[m], key_all[m]
        order = np.lexsort((s_k, key_k))
        s_k, d_k, key_k = s_k[order], d_k[order], key_k[order]
        counts_all[k] = np.bincount(key_k, minlength=NG)
        per_core.append((s_k, d_k, key_k))

    # shared tile schedule: tiles per group = max over cores
    T = (np.max(counts_all, axis=0) + 127) // 128
    tile_base = np.zeros(NG + 1, np.int64)
    tile_base[1:] = np.cumsum(T)
    t_total = int(tile_base[-1])

    # per-(stage,window) gather calls, split into <=CHUNK_T-tile chunks
    calls = []   # (window, t0, nt)
    for s in range(NSTG):
        for w in range(NW):
            g0 = (s * NW + w) * SPB
            t0 = int(tile_base[g0])
            t1 = int(tile_base[g0 + SPB]) if g0 + SPB <= NG else t_total
            t = t0
            while t < t1:
                nt = min(CHUNK_T, t1 - t)
                calls.append((w, t, nt))
                t += nt

    # per-tile metadata (shared): block + first/last within its dst block
    blk_of_tile = np.zeros(t_total, np.int64)
    first_of_tile = np.zeros(t_total, bool)
    last_of_tile = np.zeros(t_total, bool)
    for s in range(NSTG):
        for j in range(SPB):
            b = s * SPB + j
            if b >= NBLK:
                continue
            tiles = []
            for w in range(NW):
                g = (s * NW + w) * SPB + j
                tiles.extend(range(int(tile_base[g]), int(tile_base[g + 1])))
            assert tiles, f"block {b} has no tiles"
            for t in tiles:
                blk_of_tile[t] = b
            first_of_tile[tiles[0]] = True
            last_of_tile[tiles[-1]] = True

    # zero-table row per (core, window): a padded-node position in the window
    pad_pos = pos_of_node[n:]
    zero_row = np.zeros((NCORES, NW), np.int64)
    for w in range(NW):
        inw = pad_pos[(pad_pos >= w * WIN) & (pad_pos < (w + 1) * WIN)]
        assert inw.size > 0, f"no pad position in window {w}"
        zero_row[:, w] = inw[0]

    x_pad = np.zeros((NPAD, D), np.float32)
    x_pad[:n] = x
    x_perm = x_pad[node_at_pos]

    self_scale = a_pos * a_pos if bias_mode else a_pos

    in_maps = []
    for k in range(NCORES):
        s_k, d_k, key_k = per_core[k]
        ne = s_k.shape[0]
        grp_off = np.zeros(NG + 1, np.int64)
        grp_off[1:] = np.cumsum(counts_all[k])
        rank = np.arange(ne, dtype=np.int64) - grp_off[key_k]
        slot = tile_base[key_k] * 128 + rank

        gidx = np.zeros(t_total * 128, np.int64)
        dloc = np.zeros(t_total * 128, np.int64)
        adst = np.zeros(t_total * 128, np.float32)
        # default: every slot gathers the window's zero row
        tt = np.arange(t_total)
        w_of_tile = np.zeros(t_total, np.int64)
        for (w, t0, nt) in calls:
            w_of_tile[t0:t0 + nt] = w
        gidx[:] = (zero_row[k][w_of_tile[tt]] - w_of_tile[tt] * WIN).repeat(128)
        gidx[slot] = s_k - (s_k // WIN) * WIN
        dloc[slot] = d_k % 128
        adst[slot] = a_pos[d_k]

        # wrapped int16 gather indices, replicated across the 8 Q7 groups
        idxw = np.zeros((128, t_total * 8), np.int16)
        for (w, t0, nt) in calls:
            blkv = gidx[t0 * 128:(t0 + nt) * 128].astype(np.int16)
            blkv = blkv.reshape(nt * 8, 16).T
            idxw[:, t0 * 8:(t0 + nt) * 8] = np.tile(blkv, (8, 1))

        sl = slice(k * SHARD, (k + 1) * SHARD)
        xT = np.ascontiguousarray(x_perm[sl].T).astype(NPBF16)
        xT2 = np.ascontiguousarray(
            (x_perm[sl] * self_scale[sl, None]).T).astype(NPBF16)
        a_col = np.ascontiguousarray(a_pos[sl].reshape(NBLK, 128).T)
        a2_col = np.ascontiguousarray(
            (a_pos[sl] ** 2).reshape(NBLK, 128).T)
        iota = np.broadcast_to(np.arange(128, dtype=np.float32),
                               (128, 128)).astype(NPBF16).copy()
        im = {
            "xT": xT,
            "xT2": xT2,
            "W1": W1.astype(NPBF16),
            "W2": W2.astype(NPBF16),
            "a_col": a_col.astype(np.float32),
            "a2_col": a2_col.astype(np.float32),
            "iota": iota,
            "idxw": idxw,
            "dloc": np.ascontiguousarray(
                dloc.reshape(t_total, 128).T).astype(np.float32),
        }
        if bias_mode:
            im["adst"] = np.ascontiguousarray(
                adst.reshape(t_total, 128).T).astype(np.float32)
            im["b1col"] = b1.reshape(128, 1).astype(np.float32)
            im["b2bc"] = np.broadcast_to(b2, (128, 128)).astype(np.float32).copy()
        in_maps.append(im)

    sched_sig = (tuple(int(v) for v in T), tuple(calls), bias_mode)
    meta = dict(tile_base=tuple(int(v) for v in tile_base), t_total=t_total,
                calls=calls, blk_of_tile=blk_of_tile,
                first_of_tile=first_of_tile, last_of_tile=last_of_tile,
                bias_mode=bias_mode)
    return in_maps, sched_sig, meta, pos_of_node


def _build_program(meta):
    t_total = meta["t_total"]
    calls = meta["calls"]
    blk_of_tile = meta["blk_of_tile"]
    first_of_tile = meta["first_of_tile"]
    last_of_tile = meta["last_of_tile"]
    bias_mode = meta["bias_mode"]

    nc = bacc.Bacc("TRN2", target_bir_lowering=False, debug=False,
                   num_devices=NCORES, num_swdge_queues=4)
    xT_d = nc.dram_tensor("xT", [D, SHARD], BF16, kind="ExternalInput")
    xT2_d = nc.dram_tensor("xT2", [D, SHARD], BF16, kind="ExternalInput")
    W1_d = nc.dram_tensor("W1", [D, D], BF16, kind="ExternalInput")
    W2_d = nc.dram_tensor("W2", [D, D], BF16, kind="ExternalInput")
    acol_d = nc.dram_tensor("a_col", [128, NBLK], F32, kind="ExternalInput")
    a2col_d = nc.dram_tensor("a2_col", [128, NBLK], F32, kind="ExternalInput")
    iota_d = nc.dram_tensor("iota", [128, 128], BF16, kind="ExternalInput")
    idx_d = nc.dram_tensor("idxw", [128, t_total * 8], mybir.dt.int16,
                           kind="ExternalInput")
    dloc_d = nc.dram_tensor("dloc", [128, t_total], F32, kind="ExternalInput")
    if bias_mode:
        adst_d = nc.dram_tensor("adst", [128, t_total], F32,
                                kind="ExternalInput")
        b1_d = nc.dram_tensor("b1col", [128, 1], F32, kind="ExternalInput")
        b2_d = nc.dram_tensor("b2bc", [128, 128], F32, kind="ExternalInput")
    out_d = nc.dram_tensor("out", [SHARD, D], F32, kind="ExternalOutput")

    bounce1 = nc.dram_tensor("bounce1", [SHARD, D], BF16)
    bounce2 = nc.dram_tensor("bounce2", [SHARD, D], BF16)
    table1 = nc.dram_tensor("table1", [NPAD + 128, D], BF16, addr_space="Shared")
    table2 = nc.dram_tensor("table2", [NPAD + 128, D], BF16, addr_space="Shared")

    with tile.TileContext(nc) as tc:
        with (
            tc.tile_pool(name="const", bufs=1) as p_const,
            tc.tile_pool(name="big", bufs=1) as p_big,
            tc.tile_pool(name="msg", bufs=6) as p_msg,
            tc.tile_pool(name="sel", bufs=10) as p_sel,
            tc.tile_pool(name="small", bufs=4) as p_small,
            tc.tile_pool(name="psum", bufs=8, space="PSUM") as p_psum,
        ):
            W1_t = p_const.tile([D, D], BF16)
            W2_t = p_const.tile([D, D], BF16)
            iota_t = p_const.tile([128, 128], BF16)
            acol_t = p_const.tile([128, NBLK], F32)
            a2col_t = p_const.tile([128, NBLK], F32)
            idx_t = p_const.tile([128, t_total * 8], mybir.dt.int16)
            dloc_t = p_const.tile([128, t_total], F32)
            xT_t = p_const.tile([D, SHARD], BF16)
            xT2_t = p_const.tile([D, SHARD], BF16)
            for s in range(NSTG):
                sl = slice(s * SPB * 128, min((s + 1) * SPB, NBLK) * 128)
                nc.sync.dma_start(out=xT_t[:, sl], in_=xT_d[:, sl])
            for tt, dd in ((W1_t, W1_d), (W2_t, W2_d), (iota_t, iota_d),
                           (acol_t, acol_d), (a2col_t, a2col_d),
                           (idx_t, idx_d), (dloc_t, dloc_d),
                           (xT2_t, xT2_d)):
                nc.sync.dma_start(out=tt[:], in_=dd[:])
            if bias_mode:
                adst_t = p_const.tile([128, t_total], F32)
                b1_t = p_const.tile([128, 1], F32)
                b2_t = p_const.tile([128, 128], F32)
                nc.sync.dma_start(out=adst_t[:], in_=adst_d[:])
                nc.sync.dma_start(out=b1_t[:], in_=b1_d[:])
                nc.sync.dma_start(out=b2_t[:], in_=b2_d[:])

            gbuf = p_big.tile([128, SHARD], BF16, tag="gbuf")
            r1 = p_big.tile([128, SHARD], BF16, tag="r1")

            zpad = p_const.tile([128, 128], BF16)
            nc.vector.memset(zpad[:], 0.0)
            nc.sync.dma_start(out=table1[NPAD:NPAD + 128, :], in_=zpad[:])
            nc.sync.dma_start(out=table2[NPAD:NPAD + 128, :], in_=zpad[:])

            def dense(lhsT_src, W_t, scol, bounce):
                """h-blocks = lhsT.T @ W scaled per-row -> gbuf -> bounce."""
                for b in range(NBLK):
                    ps = p_psum.tile([128, D], F32, space="PSUM", tag="eps")
                    nc.tensor.matmul(out=ps[:],
                                     lhsT=lhsT_src[:, b * 128:(b + 1) * 128],
                                     rhs=W_t[:], start=True, stop=True)
                    nc.scalar.activation(
                        out=gbuf[:, b * 128:(b + 1) * 128], in_=ps[:],
                        func=mybir.ActivationFunctionType.Copy,
                        scale=scol[:, b:b + 1])
                    nc.sync.dma_start(
                        out=bounce[b * 128:(b + 1) * 128, :],
                        in_=gbuf[:, b * 128:(b + 1) * 128])

            def build_sel(t):
                S_t = p_sel.tile([128, 128], BF16, tag="sel")
                nc.vector.tensor_scalar(
                    out=S_t[:], in0=iota_t[:], scalar1=dloc_t[:, t:t + 1],
                    scalar2=None, op0=mybir.AluOpType.is_equal)
                if bias_mode:
                    nc.vector.tensor_scalar(
                        out=S_t[:], in0=S_t[:], scalar1=adst_t[:, t:t + 1],
                        scalar2=None, op0=mybir.AluOpType.mult)
                return S_t

            def edge_phase(table, transposed):
                pending = {}
                ncall = 0
                for s in range(NSTG):
                    blocks = range(s * SPB, min((s + 1) * SPB, NBLK))
                    if transposed:
                        # open each block's psum group with the self matmul
                        for b in blocks:
                            psb = p_psum.tile([128, D], F32, space="PSUM",
                                               tag="eps")
                            nc.tensor.matmul(
                                out=psb[:], lhsT=W1_t[:],
                                rhs=xT2_t[:, b * 128:(b + 1) * 128],
                                start=True, stop=False)
                            pending[b] = psb
                    # calls list is in stage order; consume by range
                    while ncall < len(calls):
                        w, t0, nt = calls[ncall]
                        if blk_of_tile[t0] // SPB != s:
                            break
                        ncall += 1
                        msg_t = p_msg.tile([128, CHUNK_T, 2 * D], BF16,
                                           tag="msg")
                        tv = table[:]
                        win_ap = bass.AP(tv.tensor, w * WIN * D,
                                         [[D, WIN], [1, 2 * D]])
                        nc.gpsimd.dma_gather(
                            out_ap=msg_t[:, :nt, :],
                            in_ap=win_ap,
                            idxs_ap=idx_t[:, t0 * 8:(t0 + nt) * 8],
                            num_idxs=nt * 128, num_idxs_reg=nt * 128,
                            elem_size=2 * D, elem_step=D, queue_num=ncall % 4)
                        for j in range(nt):
                            t = t0 + j
                            b = int(blk_of_tile[t])
                            S_t = build_sel(t)
                            if transposed:
                                psb = pending[b]
                                nc.tensor.matmul(
                                    out=psb[:], lhsT=msg_t[:, j, 0:D],
                                    rhs=S_t[:], start=False,
                                    stop=bool(last_of_tile[t]))
                            else:
                                if first_of_tile[t]:
                                    psb = p_psum.tile([128, D], F32,
                                                      space="PSUM", tag="eps")
                                    pending[b] = psb
                                psb = pending[b]
                                nc.tensor.matmul(
                                    out=psb[:], lhsT=S_t[:],
                                    rhs=msg_t[:, j, 0:D],
                                    start=bool(first_of_tile[t]),
                                    stop=bool(last_of_tile[t]))
                    # stage epilogues
                    for b in blocks:
                        psb = pending.pop(b)
                        sl = slice(b * 128, (b + 1) * 128)
                        if transposed:
                            # r1 = relu(psum) (+ b1 per-partition if biased)
                            nc.scalar.activation(
                                out=r1[:, sl], in_=psb[:],
                                func=mybir.ActivationFunctionType.Relu,
                                bias=(b1_t[:, :1] if bias_mode else 0.0))
                            # fused dense2 for this block (reuses the bank):
                            # g2 = scale * (r1.T @ W2) -> gbuf, bounce2
                            nc.tensor.matmul(out=psb[:], lhsT=r1[:, sl],
                                             rhs=W2_t[:], start=True,
                                             stop=True)
                            g2scol = acol_t if bias_mode else a2col_t
                            nc.scalar.activation(
                                out=gbuf[:, sl], in_=psb[:],
                                func=mybir.ActivationFunctionType.Copy,
                                scale=g2scol[:, b:b + 1])
                            nc.sync.dma_start(out=bounce2[sl, :],
                                              in_=gbuf[:, sl])
                        else:
                            tmp = p_small.tile([128, D], F32, tag="tmp")
                            if bias_mode:
                                # tmp = psum + a*g2 + b2 ; out = relu(tmp)
                                t2 = p_small.tile([128, D], F32, tag="t2")
                                nc.vector.tensor_scalar(
                                    out=t2[:], in0=gbuf[:, sl],
                                    scalar1=acol_t[:, b:b + 1], scalar2=None,
                                    op0=mybir.AluOpType.mult)
                                nc.vector.tensor_tensor(
                                    out=tmp[:], in0=psb[:], in1=t2[:],
                                    op=mybir.AluOpType.add)
                                nc.vector.tensor_tensor(
                                    out=tmp[:], in0=tmp[:], in1=b2_t[:],
                                    op=mybir.AluOpType.add)
                                ob = p_small.tile([128, D], F32, tag="ob")
                                nc.scalar.activation(
                                    out=ob[:], in_=tmp[:],
                                    func=mybir.ActivationFunctionType.Relu)
                            else:
                                # out = relu(a * (psum + g2))
                                nc.vector.tensor_tensor(
                                    out=tmp[:], in0=psb[:], in1=gbuf[:, sl],
                                    op=mybir.AluOpType.add)
                                ob = p_small.tile([128, D], F32, tag="ob")
                                nc.scalar.activation(
                                    out=ob[:], in_=tmp[:],
                                    func=mybir.ActivationFunctionType.Relu,
                                    scale=acol_t[:, b:b + 1])
                            nc.sync.dma_start(out=out_d[sl, :], in_=ob[:])

            # ---------- layer 1 ----------
            dense(xT_t, W1_t, acol_t, bounce1)
            nc.gpsimd.collective_compute(
                "AllGather", mybir.AluOpType.bypass,
                replica_groups=[list(range(NCORES))],
                ins=[bounce1[:]], outs=[table1[0:NPAD, :]])
            edge_phase(table1, transposed=True)

            # ---------- layer 2 (dense2 was fused into edge1 epilogues) ----
            nc.gpsimd.collective_compute(
                "AllGather", mybir.AluOpType.bypass,
                replica_groups=[list(range(NCORES))],
                ins=[bounce2[:]], outs=[table2[0:NPAD, :]])
            edge_phase(table2, transposed=False)

    nc.compile()
    return nc


def prepare(x, edge_index, W1, b1, W2, b2):
    in_maps, sched_sig, meta, pos_of_node = _host_prep(
        x, edge_index, W1, b1, W2, b2)
    if sched_sig not in _CACHE:
        _CACHE[sched_sig] = _build_program(meta)
    return _CACHE[sched_sig], in_maps, pos_of_node


def kernel(x, edge_index, W1, b1, W2, b2):
    nc, in_maps, pos_of_node = prepare(x, edge_index, W1, b1, W2, b2)
    res = run_bass_kernel_spmd(nc, in_maps, list(range(NCORES)))
    full = np.concatenate([res.results[k]["out"] for k in range(NCORES)],
                          axis=0)
    n = np.asarray(x).shape[0]
    return full[pos_of_node[:n]]
